# revision 3
# baseline (speedup 1.0000x reference)
"""Trainium2 Bass kernel for nn_Diffusion_9053791060399.

Strategy: shard the n_users axis (last dim) across 8 NeuronCores.
Each core handles 1250 users: normalizes its user-embedding slice and the
(replicated) gathered item rows on device, computes the similarity scores
with a split-precision bf16 matmul (hi/lo), forms gamma, then runs the
10-step diffusion coefficient pipeline, writing all 14 outputs.

Numerical notes:
 - 1/(1-acp) is computed as exp(-ln(1-acp)) on the scalar engine; the ln is
   shared with the log_one_minus_acp output and Ln/Exp live in one ACT
   table set (sqrt lives in another, so passes are grouped by table set).
 - 1/acp and 1/alpha are evaluated by short geometric series (their
   arguments are within 2.2% of 1, so the series is exact to f32).
"""
import sys
import numpy as np
from contextlib import ExitStack

sys.path.insert(0, "/opt/trn_rl_repo")

import concourse.bass as bass
import concourse.bacc as bacc
import concourse.tile as tile
from concourse import mybir
from concourse.bass_utils import run_bass_kernel_spmd
from concourse.masks import make_identity

f32 = mybir.dt.float32
bf16 = mybir.dt.bfloat16
i32 = mybir.dt.int32
AF = mybir.ActivationFunctionType
Alu = mybir.AluOpType

S, B, U, D, NCORES = 10, 128, 10000, 64, 8
USH = U // NCORES          # 1250 users per core
W = 625                    # chunk width (2 chunks per core)
NCH = USH // W

OUT_NAMES = [
    "sqrt_recip_alphas", "acp", "sqrt_one_minus_acp", "acp_prev", "acp_next",
    "sqrt_acp", "log_one_minus_acp", "sqrt_recip_acp", "sqrt_recipm1_acp",
    "pmc1", "pmc2", "fpc2", "fpc3", "pvar",
]


def _l2norm_rows(nc, pool, src, p, tagp):
    """Normalize rows of src[:p, :D] (f32). Returns normalized f32 tile."""
    sq = pool.tile([128, D], f32, tag=tagp + "sq")
    ss = pool.tile([128, 1], f32, tag=tagp + "ss")
    nc.scalar.activation(sq[:p], src[:p], AF.Square, accum_out=ss[:p])
    nrm = pool.tile([128, 1], f32, tag=tagp + "nrm")
    nc.scalar.activation(nrm[:p], ss[:p], AF.Sqrt)
    nc.vector.tensor_scalar_max(nrm[:p], nrm[:p], 1e-12)
    ri = pool.tile([128, 1], f32, tag=tagp + "ri")
    nc.vector.reciprocal(ri[:p], nrm[:p])
    out = pool.tile([128, D], f32, tag=tagp + "nm")
    nc.vector.tensor_scalar_mul(out[:p], src[:p], ri[:p])
    return out


def _emit(tc, ctx, u_in, it_in, int_in, bb_in, outs):
    nc = tc.nc

    consts = ctx.enter_context(tc.tile_pool(name="consts", bufs=1))
    identb = consts.tile([128, 128], bf16)
    make_identity(nc, identb)
    ones_t = consts.tile([B, W], f32)
    nc.vector.memset(ones_t, 1.0)
    zeros_t = consts.tile([B, W], f32)
    nc.vector.memset(zeros_t, 0.0)
    bb_sb = consts.tile([B, S], f32)
    bb_ap = bb_in[:]
    nc.sync.dma_start(
        out=bb_sb,
        in_=bass.AP(tensor=bb_ap.tensor, offset=bb_ap.offset, ap=[[0, B], [1, S]]),
    )
    itT_hi = consts.tile([D, B], bf16)
    itT_lo = consts.tile([D, B], bf16)
    uT_hi = consts.tile([D, USH], bf16)
    uT_lo = consts.tile([D, USH], bf16)

    # ---------- preamble: normalize + transpose embeddings ----------
    with tc.tile_pool(name="prep", bufs=3) as prep, \
         tc.tile_pool(name="tp_ps", bufs=2, space="PSUM") as tp_ps:
        it_t = prep.tile([B, D], f32)
        nc.sync.dma_start(out=it_t, in_=it_in[:, :])
        it_nm = _l2norm_rows(nc, prep, it_t, B, "it")
        it_hi = prep.tile([B, D], bf16)
        nc.vector.tensor_copy(it_hi, it_nm)
        it_lo = prep.tile([B, D], bf16)
        nc.vector.tensor_sub(it_lo, it_nm, it_hi)
        for src, dst in ((it_hi, itT_hi), (it_lo, itT_lo)):
            ps = tp_ps.tile([D, B], bf16, tag="tps")
            nc.tensor.transpose(ps, src, identb)
            nc.vector.tensor_copy(dst, ps)

        ntile = (USH + 127) // 128
        for t in range(ntile):
            r0 = t * 128
            p = min(128, USH - r0)
            u_t = prep.tile([128, D], f32, tag="u_t")
            nc.sync.dma_start(out=u_t[:p], in_=u_in[r0:r0 + p, :])
            u_nm = _l2norm_rows(nc, prep, u_t, p, "u")
            u_hi = prep.tile([128, D], bf16, tag="u_hi")
            nc.vector.tensor_copy(u_hi[:p], u_nm[:p])
            u_lo = prep.tile([128, D], bf16, tag="u_lo")
            nc.vector.tensor_sub(u_lo[:p], u_nm[:p], u_hi[:p])
            for src, dst in ((u_hi, uT_hi), (u_lo, uT_lo)):
                ps = tp_ps.tile([D, 128], bf16, tag="tps2")
                nc.tensor.transpose(ps[:, :p], src[:p], identb[:p, :p])
                nc.vector.tensor_copy(dst[:, r0:r0 + p], ps[:, :p])

    # ---------- main: per-chunk pipeline ----------
    g_pool = ctx.enter_context(tc.tile_pool(name="g_pool", bufs=2))
    sc_pool = ctx.enter_context(tc.tile_pool(name="sc_ps", bufs=2, space="PSUM"))
    acp_pool = ctx.enter_context(tc.tile_pool(name="acp_pool", bufs=6))
    om_pool = ctx.enter_context(tc.tile_pool(name="om_pool", bufs=6))
    r1m_pool = ctx.enter_context(tc.tile_pool(name="r1m_pool", bufs=3))
    wk = ctx.enter_context(tc.tile_pool(name="wk", bufs=2))
    ot = ctx.enter_context(tc.tile_pool(name="ot", bufs=2))

    def dma(name, s, w0, tile_):
        nc.sync.dma_start(out=outs[name][s, :, w0:w0 + W], in_=tile_)

    for c in range(NCH):
        w0 = c * W
        # --- scores: split-precision matmul into PSUM ---
        sc_ps = sc_pool.tile([B, W], f32, tag="sc")
        for s0, s1 in ((0, 512), (512, W)):
            rhi = uT_hi[:, w0 + s0:w0 + s1]
            rlo = uT_lo[:, w0 + s0:w0 + s1]
            o = sc_ps[:, s0:s1]
            nc.tensor.matmul(o, lhsT=itT_hi, rhs=rhi, start=True, stop=False)
            nc.tensor.matmul(o, lhsT=itT_hi, rhs=rlo, start=False, stop=False)
            nc.tensor.matmul(o, lhsT=itT_lo, rhs=rhi, start=False, stop=False)
            nc.tensor.matmul(o, lhsT=itT_lo, rhs=rlo, start=False, stop=True)

        # --- gamma = 1 - 0.01 * exp(3 * where(inter, score, -score)) ---
        int_t = g_pool.tile([B, W], i32, tag="int_t")
        nc.sync.dma_start(out=int_t, in_=int_in[:, w0:w0 + W])
        sgn = g_pool.tile([B, W], f32, tag="sgn")
        nc.vector.tensor_scalar(out=sgn, in0=int_t, scalar1=2, scalar2=-1,
                                op0=Alu.mult, op1=Alu.add)
        clip = g_pool.tile([B, W], f32, tag="clip")
        nc.vector.tensor_scalar(out=clip, in0=sc_ps, scalar1=1.0, scalar2=-1.0,
                                op0=Alu.min, op1=Alu.max)
        signed = g_pool.tile([B, W], f32, tag="signed")
        nc.vector.tensor_mul(signed, clip, sgn)
        expv = g_pool.tile([B, W], f32, tag="expv")
        nc.scalar.activation(expv, signed, AF.Exp, scale=3.0)
        gam = g_pool.tile([B, W], f32, tag="gam")
        nc.scalar.activation(gam, expv, AF.Copy, scale=-0.01, bias=1.0)

        # --- P1: cumprod chain (DVE/Pool only) + acp/prev/next outputs ---
        acp_hist, om_hist = [], []
        prev_acp = ones_t
        for s in range(S):
            beta = wk.tile([B, W], f32, tag="beta")
            nc.vector.tensor_scalar_mul(beta, gam, bb_sb[:, s:s + 1])
            alpha = wk.tile([B, W], f32, tag="alpha")
            nc.gpsimd.tensor_scalar(out=alpha, in0=beta, scalar1=-1.0,
                                    scalar2=1.0, op0=Alu.mult, op1=Alu.add)
            acp_t = acp_pool.tile([B, W], f32, tag="acp")
            nc.vector.tensor_mul(acp_t, prev_acp, alpha)
            om_t = om_pool.tile([B, W], f32, tag="om")
            nc.gpsimd.tensor_scalar(out=om_t, in0=acp_t, scalar1=-1.0,
                                    scalar2=1.0, op0=Alu.mult, op1=Alu.add)
            dma("acp", s, w0, acp_t)
            dma("acp_prev", s, w0, prev_acp)
            if s > 0:
                dma("acp_next", s - 1, w0, acp_t)
            if s == S - 1:
                dma("acp_next", s, w0, zeros_t)
            acp_hist.append(acp_t)
            om_hist.append(om_t)
            prev_acp = acp_t

        # --- P2: ln(1-acp) output + r1m = 1/(1-acp) via exp(-ln) ---
        r1m_hist = []
        for s in range(S):
            lg = ot.tile([B, W], f32, tag="lg")
            nc.scalar.activation(lg, om_hist[s], AF.Ln)
            dma("log_one_minus_acp", s, w0, lg)
            r1m = r1m_pool.tile([B, W], f32, tag="r1m")
            nc.scalar.activation(r1m, lg, AF.Exp, scale=-1.0)
            r1m_hist.append(r1m)

        # --- P3: all sqrt-set outputs ---
        sacp_prev = ones_t
        for s in range(S):
            r1m, acp_t, om_t = r1m_hist[s], acp_hist[s], om_hist[s]
            om_prev = om_hist[s - 1] if s > 0 else zeros_t

            beta = wk.tile([B, W], f32, tag="beta")
            nc.vector.tensor_scalar_mul(beta, gam, bb_sb[:, s:s + 1])
            alpha = wk.tile([B, W], f32, tag="alpha")
            nc.gpsimd.tensor_scalar(out=alpha, in0=beta, scalar1=-1.0,
                                    scalar2=1.0, op0=Alu.mult, op1=Alu.add)
            # sqrt_recip_alphas = sqrt(1/alpha); 1/alpha = 1 + b + b^2 (exact in f32)
            b1 = wk.tile([B, W], f32, tag="b1")
            nc.vector.scalar_tensor_tensor(out=b1, in0=beta, scalar=1.0,
                                           in1=beta, op0=Alu.add, op1=Alu.mult)
            sra = ot.tile([B, W], f32, tag="sra")
            nc.scalar.activation(sra, b1, AF.Sqrt, bias=1.0)
            dma("sqrt_recip_alphas", s, w0, sra)

            # 1/acp - 1 = om + om^2 + om^3 + om^4 (exact in f32)
            s1 = wk.tile([B, W], f32, tag="ser")
            nc.vector.scalar_tensor_tensor(out=s1, in0=om_t, scalar=1.0,
                                           in1=om_t, op0=Alu.add, op1=Alu.mult)
            s2 = wk.tile([B, W], f32, tag="ser")
            nc.vector.scalar_tensor_tensor(out=s2, in0=s1, scalar=1.0,
                                           in1=om_t, op0=Alu.add, op1=Alu.mult)
            s3 = wk.tile([B, W], f32, tag="ser")
            nc.vector.scalar_tensor_tensor(out=s3, in0=s2, scalar=1.0,
                                           in1=om_t, op0=Alu.add, op1=Alu.mult)
            sracp = ot.tile([B, W], f32, tag="sracp")
            nc.scalar.activation(sracp, s3, AF.Sqrt, bias=1.0)
            dma("sqrt_recip_acp", s, w0, sracp)
            srm1 = ot.tile([B, W], f32, tag="srm1")
            nc.scalar.activation(srm1, s3, AF.Sqrt)
            dma("sqrt_recipm1_acp", s, w0, srm1)

            sacp = ot.tile([B, W], f32, tag="sacp")
            nc.scalar.activation(sacp, acp_t, AF.Sqrt)
            dma("sqrt_acp", s, w0, sacp)
            s1m = ot.tile([B, W], f32, tag="s1m")
            nc.scalar.activation(s1m, om_t, AF.Sqrt)
            dma("sqrt_one_minus_acp", s, w0, s1m)
            sqa = wk.tile([B, W], f32, tag="sqa")
            nc.scalar.activation(sqa, alpha, AF.Sqrt)

            t_t = wk.tile([B, W], f32, tag="t_t")
            nc.vector.tensor_mul(t_t, beta, r1m)
            pmc1 = ot.tile([B, W], f32, tag="pmc1")
            nc.vector.tensor_mul(pmc1, t_t, sacp_prev)
            dma("pmc1", s, w0, pmc1)
            pvar = ot.tile([B, W], f32, tag="pvar")
            nc.gpsimd.tensor_mul(pvar, t_t, om_prev)
            dma("pvar", s, w0, pvar)

            u2 = wk.tile([B, W], f32, tag="u2")
            nc.vector.tensor_mul(u2, sqa, r1m)
            pmc2 = ot.tile([B, W], f32, tag="pmc2")
            nc.gpsimd.tensor_mul(pmc2, u2, om_prev)
            dma("pmc2", s, w0, pmc2)

            fpa = wk.tile([B, W], f32, tag="fpa")
            nc.gpsimd.tensor_mul(fpa, om_prev, r1m)
            fpc2 = ot.tile([B, W], f32, tag="fpc2")
            nc.scalar.activation(fpc2, fpa, AF.Sqrt)
            dma("fpc2", s, w0, fpc2)
            fpc3 = ot.tile([B, W], f32, tag="fpc3")
            nc.gpsimd.tensor_mul(fpc3, sacp, fpc2)
            dma("fpc3", s, w0, fpc3)

            sacp_prev = sacp


def _build():
    nc = bacc.Bacc()
    u_in = nc.declare_dram_parameter("u_emb", [USH, D], f32, isOutput=False)
    it_in = nc.declare_dram_parameter("it_sel", [B, D], f32, isOutput=False)
    int_in = nc.declare_dram_parameter("inter", [B, USH], i32, isOutput=False)
    bb_in = nc.declare_dram_parameter("bb", [S], f32, isOutput=False)
    outs = {
        nm: nc.declare_dram_parameter(nm, [S, B, USH], f32, isOutput=True)
        for nm in OUT_NAMES
    }
    with tile.TileContext(nc) as tc, ExitStack() as ctx:
        _emit(tc, ctx, u_in, it_in, int_in, bb_in, outs)
    nc.finalize()
    return nc


_NC = None


def _get_nc():
    global _NC
    if _NC is None:
        _NC = _build()
    return _NC


def make_in_maps(user_emb, item_emb, iids, inter, base_betas):
    user_emb = np.asarray(user_emb, dtype=np.float32)
    item_emb = np.asarray(item_emb, dtype=np.float32)
    iids = np.asarray(iids, dtype=np.int32)
    inter = np.asarray(inter, dtype=np.int32)
    base_betas = np.asarray(base_betas, dtype=np.float32)
    it_sel = np.ascontiguousarray(item_emb[iids])
    maps = []
    for c in range(NCORES):
        sl = slice(c * USH, (c + 1) * USH)
        maps.append({
            "u_emb": np.ascontiguousarray(user_emb[sl]),
            "it_sel": it_sel,
            "inter": np.ascontiguousarray(inter[:, sl]),
            "bb": base_betas,
        })
    return maps


def kernel(user_emb, item_emb, iids, inter, base_betas):
    nc = _get_nc()
    maps = make_in_maps(user_emb, item_emb, iids, inter, base_betas)
    res = run_bass_kernel_spmd(nc, maps, list(range(NCORES))).results
    return tuple(
        np.concatenate([res[c][nm] for c in range(NCORES)], axis=2)
        for nm in OUT_NAMES
    )


# revision 10
# speedup vs baseline: 1.1797x; 1.1797x over previous
"""Trainium2 Bass kernel for nn_Diffusion_9053791060399.

Strategy: shard the n_users axis (last dim) across 8 NeuronCores.
Each core handles 1250 users: normalizes its user-embedding slice and the
(replicated) gathered item rows on device, computes the similarity scores
with a split-precision bf16 matmul (hi/lo), forms gamma, then runs the
10-step diffusion coefficient pipeline, writing all 14 outputs.

Numerical notes:
 - 1/(1-acp) is computed as exp(-ln(1-acp)) on the scalar engine; the ln is
   shared with the log_one_minus_acp output and Ln/Exp live in one ACT
   table set (sqrt lives in another, so passes are grouped by table set).
 - 1/acp and 1/alpha are evaluated by short geometric series (their
   arguments are within 2.2% of 1, so the series is exact to f32).
"""
import sys
import numpy as np
from contextlib import ExitStack

sys.path.insert(0, "/opt/trn_rl_repo")

import concourse.bass as bass
import concourse.bacc as bacc
import concourse.tile as tile
from concourse import mybir
from concourse.bass_utils import run_bass_kernel_spmd
from concourse.masks import make_identity
from bass_rust import add_dep_helper

f32 = mybir.dt.float32
bf16 = mybir.dt.bfloat16
i32 = mybir.dt.int32
AF = mybir.ActivationFunctionType
Alu = mybir.AluOpType

S, B, U, D, NCORES = 10, 128, 10000, 64, 8
USH = U // NCORES          # 1250 users per core
W = 625                    # chunk width (2 chunks per core)
GRP = 5                    # steps per ACT table-set group
NCH = USH // W

OUT_NAMES = [
    "sqrt_recip_alphas", "acp", "sqrt_one_minus_acp", "acp_prev", "acp_next",
    "sqrt_acp", "log_one_minus_acp", "sqrt_recip_acp", "sqrt_recipm1_acp",
    "pmc1", "pmc2", "fpc2", "fpc3", "pvar",
]


def _l2norm_rows(nc, pool, src, p, tagp, fence=None):
    """Normalize rows of src[:p, :D] (f32). Returns normalized f32 tile."""
    sq = pool.tile([128, D], f32, tag=tagp + "sq")
    ss = pool.tile([128, 1], f32, tag=tagp + "ss")
    nc.scalar.activation(sq[:p], src[:p], AF.Square, accum_out=ss[:p])
    nrm = pool.tile([128, 1], f32, tag=tagp + "nrm")
    isq = nc.scalar.activation(nrm[:p], ss[:p], AF.Sqrt)
    if fence is not None:
        fence[0] = isq
    nc.vector.tensor_scalar_max(nrm[:p], nrm[:p], 1e-12)
    ri = pool.tile([128, 1], f32, tag=tagp + "ri")
    nc.vector.reciprocal(ri[:p], nrm[:p])
    out = pool.tile([128, D], f32, tag=tagp + "nm")
    nc.vector.tensor_scalar_mul(out[:p], src[:p], ri[:p])
    return out


def _emit(tc, ctx, u_in, it_in, int_in, bb_in, outs, reps=1):
    nc = tc.nc

    consts = ctx.enter_context(tc.tile_pool(name="consts", bufs=1))
    identb = consts.tile([128, 128], bf16)
    make_identity(nc, identb)
    ones_t = consts.tile([B, W], f32)
    nc.vector.memset(ones_t, 1.0)
    zeros_t = consts.tile([B, W], f32)
    nc.vector.memset(zeros_t, 0.0)
    bb_sb = consts.tile([B, S], f32)
    bb_ap = bb_in[:]
    nc.sync.dma_start(
        out=bb_sb,
        in_=bass.AP(tensor=bb_ap.tensor, offset=bb_ap.offset, ap=[[0, B], [1, S]]),
    )
    act_fence = [None]
    itT_hi = consts.tile([D, B], bf16)
    itT_lo = consts.tile([D, B], bf16)
    uT_hi = consts.tile([D, USH], bf16)
    uT_lo = consts.tile([D, USH], bf16)

    # ---------- preamble: normalize + transpose embeddings ----------
    with tc.tile_pool(name="prep", bufs=3) as prep, \
         tc.tile_pool(name="tp_ps", bufs=2, space="PSUM") as tp_ps:
        it_t = prep.tile([B, D], f32)
        nc.sync.dma_start(out=it_t, in_=it_in[:, :])
        it_nm = _l2norm_rows(nc, prep, it_t, B, "it", fence=act_fence)
        it_hi = prep.tile([B, D], bf16)
        nc.vector.tensor_copy(it_hi, it_nm)
        it_lo = prep.tile([B, D], bf16)
        nc.vector.tensor_sub(it_lo, it_nm, it_hi)
        for src, dst in ((it_hi, itT_hi), (it_lo, itT_lo)):
            ps = tp_ps.tile([D, B], bf16, tag="tps")
            nc.tensor.transpose(ps, src, identb)
            nc.vector.tensor_copy(dst, ps)

        ntile = (USH + 127) // 128
        for t in range(ntile):
            r0 = t * 128
            p = min(128, USH - r0)
            u_t = prep.tile([128, D], f32, tag="u_t")
            nc.sync.dma_start(out=u_t[:p], in_=u_in[r0:r0 + p, :])
            u_nm = _l2norm_rows(nc, prep, u_t, p, "u", fence=act_fence)
            u_hi = prep.tile([128, D], bf16, tag="u_hi")
            nc.vector.tensor_copy(u_hi[:p], u_nm[:p])
            u_lo = prep.tile([128, D], bf16, tag="u_lo")
            nc.vector.tensor_sub(u_lo[:p], u_nm[:p], u_hi[:p])
            for src, dst in ((u_hi, uT_hi), (u_lo, uT_lo)):
                ps = tp_ps.tile([D, 128], bf16, tag="tps2")
                nc.tensor.transpose(ps[:, :p], src[:p], identb[:p, :p])
                nc.vector.tensor_copy(dst[:, r0:r0 + p], ps[:, :p])

    # ---------- main: per-chunk pipeline ----------
    g_pool = ctx.enter_context(tc.tile_pool(name="g_pool", bufs=2))
    sc_pool = ctx.enter_context(tc.tile_pool(name="sc_ps", bufs=2, space="PSUM"))
    acp_pool = ctx.enter_context(tc.tile_pool(name="acp_pool", bufs=8))
    om_pool = ctx.enter_context(tc.tile_pool(name="om_pool", bufs=8))
    r1m_pool = ctx.enter_context(tc.tile_pool(name="r1m_pool", bufs=GRP))
    wk = ctx.enter_context(tc.tile_pool(name="wk", bufs=2))
    ot = ctx.enter_context(tc.tile_pool(name="ot", bufs=2))

    def dma(name, s, w0, tile_):
        nc.sync.dma_start(out=outs[name][s, :, w0:w0 + W], in_=tile_)

    def dma_pool(name, s, w0, tile_):
        nc.gpsimd.dma_start(out=outs[name][s, :, w0:w0 + W], in_=tile_)

    def dma_act(name, s, w0, tile_):
        nc.scalar.dma_start(out=outs[name][s, :, w0:w0 + W], in_=tile_)

    for rep in range(reps):
      gams = []
      for c in range(NCH):
        w0 = c * W
        # --- scores: split-precision matmul into PSUM ---
        sc_ps = sc_pool.tile([B, W], f32, tag="sc")
        for s0, s1 in ((0, 512), (512, W)):
            rhi = uT_hi[:, w0 + s0:w0 + s1]
            rlo = uT_lo[:, w0 + s0:w0 + s1]
            o = sc_ps[:, s0:s1]
            nc.tensor.matmul(o, lhsT=itT_hi, rhs=rhi, start=True, stop=False)
            nc.tensor.matmul(o, lhsT=itT_hi, rhs=rlo, start=False, stop=False)
            nc.tensor.matmul(o, lhsT=itT_lo, rhs=rhi, start=False, stop=False)
            nc.tensor.matmul(o, lhsT=itT_lo, rhs=rlo, start=False, stop=True)

        # --- gamma = 1 - 0.01 * exp(3 * where(inter, score, -score)) ---
        int_t = g_pool.tile([B, W], i32, tag="int_t", bufs=1)
        nc.sync.dma_start(out=int_t, in_=int_in[:, w0:w0 + W])
        sgn = g_pool.tile([B, W], f32, tag="sgn", bufs=1)
        nc.vector.tensor_scalar(out=sgn, in0=int_t, scalar1=2, scalar2=-1,
                                op0=Alu.mult, op1=Alu.add)
        clip = g_pool.tile([B, W], f32, tag="clip", bufs=1)
        nc.vector.tensor_scalar(out=clip, in0=sc_ps, scalar1=1.0, scalar2=-1.0,
                                op0=Alu.min, op1=Alu.max)
        signed = g_pool.tile([B, W], f32, tag="signed", bufs=1)
        nc.vector.tensor_mul(signed, clip, sgn)
        expv = g_pool.tile([B, W], f32, tag="expv", bufs=1)
        iexp = nc.scalar.activation(expv, signed, AF.Exp, scale=3.0)
        if act_fence[0] is not None:
            add_dep_helper(iexp.ins, act_fence[0].ins, False, "act set order")
        gam = g_pool.tile([B, W], f32, tag="gam")
        nc.gpsimd.tensor_scalar(out=gam, in0=expv, scalar1=-0.01, scalar2=1.0,
                                op0=Alu.mult, op1=Alu.add)
        gams.append(gam)

      for c in range(NCH):
        w0 = c * W
        gam = gams[c]
        # --- P1: cumprod chain (DVE/Pool only) + acp/prev/next outputs ---
        acp_hist, om_hist = [], []
        prev_acp = ones_t
        for s in range(S):
            beta = wk.tile([B, W], f32, tag="beta")
            nc.vector.tensor_scalar_mul(beta, gam, bb_sb[:, s:s + 1])
            alpha = wk.tile([B, W], f32, tag="alpha")
            nc.gpsimd.tensor_scalar(out=alpha, in0=beta, scalar1=-1.0,
                                    scalar2=1.0, op0=Alu.mult, op1=Alu.add)
            acp_t = acp_pool.tile([B, W], f32, tag="acp")
            nc.vector.tensor_mul(acp_t, prev_acp, alpha)
            om_t = om_pool.tile([B, W], f32, tag="om")
            nc.gpsimd.tensor_scalar(out=om_t, in0=acp_t, scalar1=-1.0,
                                    scalar2=1.0, op0=Alu.mult, op1=Alu.add)
            dma("acp", s, w0, acp_t)
            dma_pool("acp_prev", s, w0, prev_acp)
            if s > 0:
                dma("acp_next", s - 1, w0, acp_t)
            if s == S - 1:
                dma("acp_next", s, w0, zeros_t)
            acp_hist.append(acp_t)
            om_hist.append(om_t)
            prev_acp = acp_t

        # --- P2/P3 in groups of 5 steps; ACT phases strictly ordered so
        # the Ln/Exp table set and the Sqrt table set never interleave ---
        sacp_prev = ones_t
        for g0 in range(0, S, GRP):
            gsteps = range(g0, min(g0 + GRP, S))
            # P2: ln(1-acp) output + r1m = 1/(1-acp) via exp(-ln)
            r1m_hist = {}
            p2_insts = []
            for s in gsteps:
                lg = ot.tile([B, W], f32, tag="lg", bufs=5)
                i1 = nc.scalar.activation(lg, om_hist[s], AF.Ln)
                dma("log_one_minus_acp", s, w0, lg)
                r1m = r1m_pool.tile([B, W], f32, tag="r1m")
                i2 = nc.scalar.activation(r1m, lg, AF.Exp, scale=-1.0)
                r1m_hist[s] = r1m
                if act_fence[0] is not None:
                    add_dep_helper(i1.ins, act_fence[0].ins, False, "act set order")
                    add_dep_helper(i2.ins, act_fence[0].ins, False, "act set order")
                p2_insts.extend([i1, i2])
            fence = p2_insts[-1]
            p3_last = None

            # P3: all sqrt-set outputs for this group
            for s in gsteps:
                r1m, acp_t, om_t = r1m_hist[s], acp_hist[s], om_hist[s]
                om_prev = om_hist[s - 1] if s > 0 else zeros_t

                def act(out_, in_, func, **kw):
                    nonlocal p3_last
                    i = nc.scalar.activation(out_, in_, func, **kw)
                    add_dep_helper(i.ins, fence.ins, False, "act set order")
                    p3_last = i
                    return i

                beta = wk.tile([B, W], f32, tag="beta")
                nc.vector.tensor_scalar_mul(beta, gam, bb_sb[:, s:s + 1])
                alpha = wk.tile([B, W], f32, tag="alpha")
                nc.gpsimd.tensor_scalar(out=alpha, in0=beta, scalar1=-1.0,
                                        scalar2=1.0, op0=Alu.mult, op1=Alu.add)
                # sqrt_recip_alphas: 1/alpha = 1 + b + b^2 (exact in f32)
                b1 = wk.tile([B, W], f32, tag="b1")
                nc.vector.scalar_tensor_tensor(out=b1, in0=beta, scalar=1.0,
                                               in1=beta, op0=Alu.add, op1=Alu.mult)
                sra = ot.tile([B, W], f32, tag="sra", bufs=3)
                act(sra, b1, AF.Sqrt, bias=1.0)
                dma("sqrt_recip_alphas", s, w0, sra)

                # 1/acp - 1 = om + om^2 + om^3 + om^4 (exact in f32)
                s1 = wk.tile([B, W], f32, tag="ser")
                nc.vector.scalar_tensor_tensor(out=s1, in0=om_t, scalar=1.0,
                                               in1=om_t, op0=Alu.add, op1=Alu.mult)
                s2 = wk.tile([B, W], f32, tag="ser")
                nc.vector.scalar_tensor_tensor(out=s2, in0=s1, scalar=1.0,
                                               in1=om_t, op0=Alu.add, op1=Alu.mult)
                s3 = wk.tile([B, W], f32, tag="ser")
                nc.vector.scalar_tensor_tensor(out=s3, in0=s2, scalar=1.0,
                                               in1=om_t, op0=Alu.add, op1=Alu.mult)
                sracp = ot.tile([B, W], f32, tag="sracp")
                act(sracp, s3, AF.Sqrt, bias=1.0)
                dma_pool("sqrt_recip_acp", s, w0, sracp)
                srm1 = ot.tile([B, W], f32, tag="srm1")
                act(srm1, s3, AF.Sqrt)
                dma_pool("sqrt_recipm1_acp", s, w0, srm1)

                sacp = ot.tile([B, W], f32, tag="sacp")
                act(sacp, acp_t, AF.Sqrt)
                dma_act("sqrt_acp", s, w0, sacp)
                s1m = ot.tile([B, W], f32, tag="s1m", bufs=3)
                act(s1m, om_t, AF.Sqrt)
                dma("sqrt_one_minus_acp", s, w0, s1m)
                sqa = wk.tile([B, W], f32, tag="sqa")
                act(sqa, alpha, AF.Sqrt)

                t_t = wk.tile([B, W], f32, tag="t_t")
                nc.vector.tensor_mul(t_t, beta, r1m)
                pmc1 = ot.tile([B, W], f32, tag="pmc1", bufs=3)
                nc.vector.tensor_mul(pmc1, t_t, sacp_prev)
                dma("pmc1", s, w0, pmc1)
                pvar = ot.tile([B, W], f32, tag="pvar")
                nc.gpsimd.tensor_mul(pvar, t_t, om_prev)
                dma("pvar", s, w0, pvar)

                u2 = wk.tile([B, W], f32, tag="u2")
                nc.vector.tensor_mul(u2, sqa, r1m)
                pmc2 = ot.tile([B, W], f32, tag="pmc2")
                nc.gpsimd.tensor_mul(pmc2, u2, om_prev)
                dma_pool("pmc2", s, w0, pmc2)

                fpa = wk.tile([B, W], f32, tag="fpa")
                nc.gpsimd.tensor_mul(fpa, om_prev, r1m)
                fpc2 = ot.tile([B, W], f32, tag="fpc2", bufs=3)
                act(fpc2, fpa, AF.Sqrt)
                dma("fpc2", s, w0, fpc2)
                fpc3 = ot.tile([B, W], f32, tag="fpc3", bufs=3)
                nc.gpsimd.tensor_mul(fpc3, sacp, fpc2)
                dma_pool("fpc3", s, w0, fpc3)

                sacp_prev = sacp
            act_fence[0] = p3_last


class _Bacc(bacc.Bacc):
    """Bacc whose ACT table-load pass maps Exp and Ln to the combined
    natural_log_exp_and_others set (the default first-match picks two
    different sets, causing a ~2.7us table reload per Ln/Exp pair)."""

    def insert_act_table_loads(self):
        has_activation = any(
            isinstance(i, mybir.InstActivation)
            for b in self.main_func.blocks
            for i in b.instructions
        )
        if not has_activation:
            return
        from concourse.hw_specs import get_activation_tables
        import bass_rust as _bass_rust
        tables = []
        for name, funcs in get_activation_tables(self.m.arch).items():
            if name != "natural_log_exp_and_others":
                funcs = funcs - {AF.Exp, AF.Ln}
            tables.append((name, funcs))
        _bass_rust.insert_act_table_loads(self, tables)


def _build(reps=1):
    nc = _Bacc()
    u_in = nc.declare_dram_parameter("u_emb", [USH, D], f32, isOutput=False)
    it_in = nc.declare_dram_parameter("it_sel", [B, D], f32, isOutput=False)
    int_in = nc.declare_dram_parameter("inter", [B, USH], i32, isOutput=False)
    bb_in = nc.declare_dram_parameter("bb", [S], f32, isOutput=False)
    outs = {
        nm: nc.declare_dram_parameter(nm, [S, B, USH], f32, isOutput=True)
        for nm in OUT_NAMES
    }
    with tile.TileContext(nc) as tc, ExitStack() as ctx:
        _emit(tc, ctx, u_in, it_in, int_in, bb_in, outs, reps=reps)
    nc.finalize()
    return nc


_NC = None


def _get_nc():
    global _NC
    if _NC is None:
        _NC = _build()
    return _NC


def make_in_maps(user_emb, item_emb, iids, inter, base_betas):
    user_emb = np.asarray(user_emb, dtype=np.float32)
    item_emb = np.asarray(item_emb, dtype=np.float32)
    iids = np.asarray(iids, dtype=np.int32)
    inter = np.asarray(inter, dtype=np.int32)
    base_betas = np.asarray(base_betas, dtype=np.float32)
    it_sel = np.ascontiguousarray(item_emb[iids])
    maps = []
    for c in range(NCORES):
        sl = slice(c * USH, (c + 1) * USH)
        maps.append({
            "u_emb": np.ascontiguousarray(user_emb[sl]),
            "it_sel": it_sel,
            "inter": np.ascontiguousarray(inter[:, sl]),
            "bb": base_betas,
        })
    return maps


def kernel(user_emb, item_emb, iids, inter, base_betas):
    nc = _get_nc()
    maps = make_in_maps(user_emb, item_emb, iids, inter, base_betas)
    res = run_bass_kernel_spmd(nc, maps, list(range(NCORES))).results
    return tuple(
        np.concatenate([res[c][nm] for c in range(NCORES)], axis=2)
        for nm in OUT_NAMES
    )


# revision 11
# speedup vs baseline: 359.5443x; 304.7775x over previous
"""Trainium2 Bass kernel for nn_Diffusion_9053791060399.

Strategy: shard the n_users axis (last dim) across 8 NeuronCores.
Each core handles 1250 users: normalizes its user-embedding slice and the
(replicated) gathered item rows on device, computes the similarity scores
with a split-precision bf16 matmul (hi/lo), forms gamma, then runs the
10-step diffusion coefficient pipeline, writing all 14 outputs.

Numerical notes:
 - 1/(1-acp) is computed as exp(-ln(1-acp)) on the scalar engine; the ln is
   shared with the log_one_minus_acp output and Ln/Exp live in one ACT
   table set (sqrt lives in another, so passes are grouped by table set).
 - 1/acp and 1/alpha are evaluated by short geometric series (their
   arguments are within 2.2% of 1, so the series is exact to f32).
"""
import sys
import numpy as np
from contextlib import ExitStack

sys.path.insert(0, "/opt/trn_rl_repo")

import concourse.bass as bass
import concourse.bacc as bacc
import concourse.tile as tile
from concourse import mybir
from concourse.bass_utils import run_bass_kernel_spmd
from concourse.masks import make_identity
from bass_rust import add_dep_helper

f32 = mybir.dt.float32
bf16 = mybir.dt.bfloat16
i32 = mybir.dt.int32
AF = mybir.ActivationFunctionType
Alu = mybir.AluOpType

S, B, U, D, NCORES = 10, 128, 10000, 64, 8
USH = U // NCORES          # 1250 users per core
W = 625                    # chunk width (2 chunks per core)
GRP = 5                    # steps per ACT table-set group
NCH = USH // W

OUT_NAMES = [
    "sqrt_recip_alphas", "acp", "sqrt_one_minus_acp", "acp_prev", "acp_next",
    "sqrt_acp", "log_one_minus_acp", "sqrt_recip_acp", "sqrt_recipm1_acp",
    "pmc1", "pmc2", "fpc2", "fpc3", "pvar",
]


def _l2norm_rows(nc, pool, src, p, tagp, fence=None):
    """Normalize rows of src[:p, :D] (f32). Returns normalized f32 tile."""
    sq = pool.tile([128, D], f32, tag=tagp + "sq")
    ss = pool.tile([128, 1], f32, tag=tagp + "ss")
    nc.scalar.activation(sq[:p], src[:p], AF.Square, accum_out=ss[:p])
    nrm = pool.tile([128, 1], f32, tag=tagp + "nrm")
    isq = nc.scalar.activation(nrm[:p], ss[:p], AF.Sqrt)
    if fence is not None:
        fence[0] = isq
    nc.vector.tensor_scalar_max(nrm[:p], nrm[:p], 1e-12)
    ri = pool.tile([128, 1], f32, tag=tagp + "ri")
    nc.vector.reciprocal(ri[:p], nrm[:p])
    out = pool.tile([128, D], f32, tag=tagp + "nm")
    nc.vector.tensor_scalar_mul(out[:p], src[:p], ri[:p])
    return out


def _emit(tc, ctx, u_in, it_in, int_in, bb_in, outs, reps=1):
    nc = tc.nc

    consts = ctx.enter_context(tc.tile_pool(name="consts", bufs=1))
    identb = consts.tile([128, 128], bf16)
    make_identity(nc, identb)
    ones_t = consts.tile([B, W], f32)
    nc.vector.memset(ones_t, 1.0)
    zeros_t = consts.tile([B, W], f32)
    nc.vector.memset(zeros_t, 0.0)
    bb_sb = consts.tile([B, S], f32)
    bb_ap = bb_in[:]
    nc.sync.dma_start(
        out=bb_sb,
        in_=bass.AP(tensor=bb_ap.tensor, offset=bb_ap.offset, ap=[[0, B], [1, S]]),
    )
    act_fence = [None]
    itT_hi = consts.tile([D, B], bf16)
    itT_lo = consts.tile([D, B], bf16)
    uT_hi = consts.tile([D, USH], bf16)
    uT_lo = consts.tile([D, USH], bf16)

    # ---------- preamble: normalize + transpose embeddings ----------
    with tc.tile_pool(name="prep", bufs=3) as prep, \
         tc.tile_pool(name="tp_ps", bufs=2, space="PSUM") as tp_ps:
        it_t = prep.tile([B, D], f32)
        nc.sync.dma_start(out=it_t, in_=it_in[:, :])
        it_nm = _l2norm_rows(nc, prep, it_t, B, "it", fence=act_fence)
        it_hi = prep.tile([B, D], bf16)
        nc.vector.tensor_copy(it_hi, it_nm)
        it_lo = prep.tile([B, D], bf16)
        nc.vector.tensor_sub(it_lo, it_nm, it_hi)
        for src, dst in ((it_hi, itT_hi), (it_lo, itT_lo)):
            ps = tp_ps.tile([D, B], bf16, tag="tps")
            nc.tensor.transpose(ps, src, identb)
            nc.vector.tensor_copy(dst, ps)

        ntile = (USH + 127) // 128
        for t in range(ntile):
            r0 = t * 128
            p = min(128, USH - r0)
            u_t = prep.tile([128, D], f32, tag="u_t")
            nc.sync.dma_start(out=u_t[:p], in_=u_in[r0:r0 + p, :])
            u_nm = _l2norm_rows(nc, prep, u_t, p, "u", fence=act_fence)
            u_hi = prep.tile([128, D], bf16, tag="u_hi")
            nc.vector.tensor_copy(u_hi[:p], u_nm[:p])
            u_lo = prep.tile([128, D], bf16, tag="u_lo")
            nc.vector.tensor_sub(u_lo[:p], u_nm[:p], u_hi[:p])
            for src, dst in ((u_hi, uT_hi), (u_lo, uT_lo)):
                ps = tp_ps.tile([D, 128], bf16, tag="tps2")
                nc.tensor.transpose(ps[:, :p], src[:p], identb[:p, :p])
                nc.vector.tensor_copy(dst[:, r0:r0 + p], ps[:, :p])

    # ---------- main: per-chunk pipeline ----------
    g_pool = ctx.enter_context(tc.tile_pool(name="g_pool", bufs=2))
    sc_pool = ctx.enter_context(tc.tile_pool(name="sc_ps", bufs=2, space="PSUM"))
    acp_pool = ctx.enter_context(tc.tile_pool(name="acp_pool", bufs=8))
    om_pool = ctx.enter_context(tc.tile_pool(name="om_pool", bufs=8))
    r1m_pool = ctx.enter_context(tc.tile_pool(name="r1m_pool", bufs=GRP))
    wk = ctx.enter_context(tc.tile_pool(name="wk", bufs=2))
    ot = ctx.enter_context(tc.tile_pool(name="ot", bufs=2))

    def dma(name, s, w0, tile_):
        nc.sync.dma_start(out=outs[name][s, :, w0:w0 + W], in_=tile_)

    def dma_pool(name, s, w0, tile_):
        nc.gpsimd.dma_start(out=outs[name][s, :, w0:w0 + W], in_=tile_)

    def dma_act(name, s, w0, tile_):
        nc.scalar.dma_start(out=outs[name][s, :, w0:w0 + W], in_=tile_)

    for rep in range(reps):
      gams = []
      for c in range(NCH):
        w0 = c * W
        # --- scores: split-precision matmul into PSUM ---
        sc_ps = sc_pool.tile([B, W], f32, tag="sc")
        for s0, s1 in ((0, 512), (512, W)):
            rhi = uT_hi[:, w0 + s0:w0 + s1]
            rlo = uT_lo[:, w0 + s0:w0 + s1]
            o = sc_ps[:, s0:s1]
            nc.tensor.matmul(o, lhsT=itT_hi, rhs=rhi, start=True, stop=False)
            nc.tensor.matmul(o, lhsT=itT_hi, rhs=rlo, start=False, stop=False)
            nc.tensor.matmul(o, lhsT=itT_lo, rhs=rhi, start=False, stop=False)
            nc.tensor.matmul(o, lhsT=itT_lo, rhs=rlo, start=False, stop=True)

        # --- gamma = 1 - 0.01 * exp(3 * where(inter, score, -score)) ---
        int_t = g_pool.tile([B, W], i32, tag="int_t", bufs=1)
        nc.sync.dma_start(out=int_t, in_=int_in[:, w0:w0 + W])
        sgn = g_pool.tile([B, W], f32, tag="sgn", bufs=1)
        nc.vector.tensor_scalar(out=sgn, in0=int_t, scalar1=2, scalar2=-1,
                                op0=Alu.mult, op1=Alu.add)
        clip = g_pool.tile([B, W], f32, tag="clip", bufs=1)
        nc.vector.tensor_scalar(out=clip, in0=sc_ps, scalar1=1.0, scalar2=-1.0,
                                op0=Alu.min, op1=Alu.max)
        signed = g_pool.tile([B, W], f32, tag="signed", bufs=1)
        nc.vector.tensor_mul(signed, clip, sgn)
        expv = g_pool.tile([B, W], f32, tag="expv", bufs=1)
        iexp = nc.scalar.activation(expv, signed, AF.Exp, scale=3.0)
        if act_fence[0] is not None:
            add_dep_helper(iexp.ins, act_fence[0].ins, False, "act set order")
        gam = g_pool.tile([B, W], f32, tag="gam")
        nc.gpsimd.tensor_scalar(out=gam, in0=expv, scalar1=-0.01, scalar2=1.0,
                                op0=Alu.mult, op1=Alu.add)
        gams.append(gam)

      for c in range(NCH):
        w0 = c * W
        gam = gams[c]
        # --- P1: cumprod chain (DVE/Pool only) + acp/prev/next outputs ---
        acp_hist, om_hist = [], []
        prev_acp = ones_t
        for s in range(S):
            beta = wk.tile([B, W], f32, tag="beta")
            nc.vector.tensor_scalar_mul(beta, gam, bb_sb[:, s:s + 1])
            alpha = wk.tile([B, W], f32, tag="alpha")
            nc.gpsimd.tensor_scalar(out=alpha, in0=beta, scalar1=-1.0,
                                    scalar2=1.0, op0=Alu.mult, op1=Alu.add)
            acp_t = acp_pool.tile([B, W], f32, tag="acp")
            nc.vector.tensor_mul(acp_t, prev_acp, alpha)
            om_t = om_pool.tile([B, W], f32, tag="om")
            nc.gpsimd.tensor_scalar(out=om_t, in0=acp_t, scalar1=-1.0,
                                    scalar2=1.0, op0=Alu.mult, op1=Alu.add)
            dma("acp", s, w0, acp_t)
            dma_pool("acp_prev", s, w0, prev_acp)
            if s > 0:
                dma("acp_next", s - 1, w0, acp_t)
            if s == S - 1:
                dma("acp_next", s, w0, zeros_t)
            acp_hist.append(acp_t)
            om_hist.append(om_t)
            prev_acp = acp_t

        # --- P2/P3 in groups of 5 steps; ACT phases strictly ordered so
        # the Ln/Exp table set and the Sqrt table set never interleave ---
        sacp_prev = ones_t
        for g0 in range(0, S, GRP):
            gsteps = range(g0, min(g0 + GRP, S))
            # P2: ln(1-acp) output + r1m = 1/(1-acp) via exp(-ln)
            r1m_hist = {}
            p2_insts = []
            for s in gsteps:
                lg = ot.tile([B, W], f32, tag="lg", bufs=5)
                i1 = nc.scalar.activation(lg, om_hist[s], AF.Ln)
                dma("log_one_minus_acp", s, w0, lg)
                r1m = r1m_pool.tile([B, W], f32, tag="r1m")
                i2 = nc.scalar.activation(r1m, lg, AF.Exp, scale=-1.0)
                r1m_hist[s] = r1m
                if act_fence[0] is not None:
                    add_dep_helper(i1.ins, act_fence[0].ins, False, "act set order")
                    add_dep_helper(i2.ins, act_fence[0].ins, False, "act set order")
                p2_insts.extend([i1, i2])
            fence = p2_insts[-1]
            p3_last = None

            # P3: all sqrt-set outputs for this group
            for s in gsteps:
                r1m, acp_t, om_t = r1m_hist[s], acp_hist[s], om_hist[s]
                om_prev = om_hist[s - 1] if s > 0 else zeros_t

                def act(out_, in_, func, **kw):
                    nonlocal p3_last
                    i = nc.scalar.activation(out_, in_, func, **kw)
                    add_dep_helper(i.ins, fence.ins, False, "act set order")
                    p3_last = i
                    return i

                beta = wk.tile([B, W], f32, tag="beta")
                nc.vector.tensor_scalar_mul(beta, gam, bb_sb[:, s:s + 1])
                alpha = wk.tile([B, W], f32, tag="alpha")
                nc.gpsimd.tensor_scalar(out=alpha, in0=beta, scalar1=-1.0,
                                        scalar2=1.0, op0=Alu.mult, op1=Alu.add)
                # sqrt_recip_alphas: 1/alpha = 1 + b + b^2 (exact in f32)
                b1 = wk.tile([B, W], f32, tag="b1")
                nc.vector.scalar_tensor_tensor(out=b1, in0=beta, scalar=1.0,
                                               in1=beta, op0=Alu.add, op1=Alu.mult)
                sra = ot.tile([B, W], f32, tag="sra", bufs=3)
                act(sra, b1, AF.Sqrt, bias=1.0)
                dma("sqrt_recip_alphas", s, w0, sra)

                # 1/acp - 1 = om + om^2 + om^3 + om^4 (exact in f32)
                s1 = wk.tile([B, W], f32, tag="ser")
                nc.vector.scalar_tensor_tensor(out=s1, in0=om_t, scalar=1.0,
                                               in1=om_t, op0=Alu.add, op1=Alu.mult)
                s2 = wk.tile([B, W], f32, tag="ser")
                nc.vector.scalar_tensor_tensor(out=s2, in0=s1, scalar=1.0,
                                               in1=om_t, op0=Alu.add, op1=Alu.mult)
                s3 = wk.tile([B, W], f32, tag="ser")
                nc.vector.scalar_tensor_tensor(out=s3, in0=s2, scalar=1.0,
                                               in1=om_t, op0=Alu.add, op1=Alu.mult)
                sracp = ot.tile([B, W], f32, tag="sracp")
                act(sracp, s3, AF.Sqrt, bias=1.0)
                dma_pool("sqrt_recip_acp", s, w0, sracp)
                srm1 = ot.tile([B, W], f32, tag="srm1")
                act(srm1, s3, AF.Sqrt)
                dma_pool("sqrt_recipm1_acp", s, w0, srm1)

                sacp = ot.tile([B, W], f32, tag="sacp")
                act(sacp, acp_t, AF.Sqrt)
                dma_act("sqrt_acp", s, w0, sacp)
                s1m = ot.tile([B, W], f32, tag="s1m", bufs=3)
                act(s1m, om_t, AF.Sqrt)
                dma("sqrt_one_minus_acp", s, w0, s1m)
                sqa = wk.tile([B, W], f32, tag="sqa")
                act(sqa, alpha, AF.Sqrt)

                t_t = wk.tile([B, W], f32, tag="t_t")
                nc.vector.tensor_mul(t_t, beta, r1m)
                pmc1 = ot.tile([B, W], f32, tag="pmc1", bufs=3)
                nc.vector.tensor_mul(pmc1, t_t, sacp_prev)
                dma("pmc1", s, w0, pmc1)
                pvar = ot.tile([B, W], f32, tag="pvar")
                nc.gpsimd.tensor_mul(pvar, t_t, om_prev)
                dma("pvar", s, w0, pvar)

                u2 = wk.tile([B, W], f32, tag="u2")
                nc.vector.tensor_mul(u2, sqa, r1m)
                pmc2 = ot.tile([B, W], f32, tag="pmc2")
                nc.gpsimd.tensor_mul(pmc2, u2, om_prev)
                dma_pool("pmc2", s, w0, pmc2)

                fpa = wk.tile([B, W], f32, tag="fpa")
                nc.gpsimd.tensor_mul(fpa, om_prev, r1m)
                fpc2 = ot.tile([B, W], f32, tag="fpc2", bufs=3)
                act(fpc2, fpa, AF.Sqrt)
                dma("fpc2", s, w0, fpc2)
                fpc3 = ot.tile([B, W], f32, tag="fpc3", bufs=3)
                nc.vector.tensor_mul(fpc3, sacp, fpc2)
                dma_pool("fpc3", s, w0, fpc3)

                sacp_prev = sacp
            act_fence[0] = p3_last


class _Bacc(bacc.Bacc):
    """Bacc whose ACT table-load pass maps Exp and Ln to the combined
    natural_log_exp_and_others set (the default first-match picks two
    different sets, causing a ~2.7us table reload per Ln/Exp pair)."""

    def insert_act_table_loads(self):
        has_activation = any(
            isinstance(i, mybir.InstActivation)
            for b in self.main_func.blocks
            for i in b.instructions
        )
        if not has_activation:
            return
        from concourse.hw_specs import get_activation_tables
        import bass_rust as _bass_rust
        tables = []
        for name, funcs in get_activation_tables(self.m.arch).items():
            if name != "natural_log_exp_and_others":
                funcs = funcs - {AF.Exp, AF.Ln}
            tables.append((name, funcs))
        _bass_rust.insert_act_table_loads(self, tables)


def _build(reps=1):
    nc = _Bacc()
    u_in = nc.declare_dram_parameter("u_emb", [USH, D], f32, isOutput=False)
    it_in = nc.declare_dram_parameter("it_sel", [B, D], f32, isOutput=False)
    int_in = nc.declare_dram_parameter("inter", [B, USH], i32, isOutput=False)
    bb_in = nc.declare_dram_parameter("bb", [S], f32, isOutput=False)
    outs = {
        nm: nc.declare_dram_parameter(nm, [S, B, USH], f32, isOutput=True)
        for nm in OUT_NAMES
    }
    with tile.TileContext(nc) as tc, ExitStack() as ctx:
        _emit(tc, ctx, u_in, it_in, int_in, bb_in, outs, reps=reps)
    nc.finalize()
    return nc


_NC = None


def _get_nc():
    global _NC
    if _NC is None:
        _NC = _build()
    return _NC


def make_in_maps(user_emb, item_emb, iids, inter, base_betas):
    user_emb = np.asarray(user_emb, dtype=np.float32)
    item_emb = np.asarray(item_emb, dtype=np.float32)
    iids = np.asarray(iids, dtype=np.int32)
    inter = np.asarray(inter, dtype=np.int32)
    base_betas = np.asarray(base_betas, dtype=np.float32)
    it_sel = np.ascontiguousarray(item_emb[iids])
    maps = []
    for c in range(NCORES):
        sl = slice(c * USH, (c + 1) * USH)
        maps.append({
            "u_emb": np.ascontiguousarray(user_emb[sl]),
            "it_sel": it_sel,
            "inter": np.ascontiguousarray(inter[:, sl]),
            "bb": base_betas,
        })
    return maps


def kernel(user_emb, item_emb, iids, inter, base_betas):
    nc = _get_nc()
    maps = make_in_maps(user_emb, item_emb, iids, inter, base_betas)
    res = run_bass_kernel_spmd(nc, maps, list(range(NCORES))).results
    return tuple(
        np.concatenate([res[c][nm] for c in range(NCORES)], axis=2)
        for nm in OUT_NAMES
    )
